# revision 9
# baseline (speedup 1.0000x reference)
"""CosArcLoss on 8 TRN2 NeuronCores (Bass/Tile).

Math (reference, f32):
    t_i   = preds[i, labels[i]]
    theta = arccos(clip(t_i, -1+1e-12, 1-1e-12))    # == clip(t_i,-1,1) in f32
    num_i = 30*(cos(theta + 0.5) - 0.35)
          = 30*cos(0.5)*t_i - 30*sin(0.5)*sqrt(1-t_i^2) - 10.5
    S_i   = sum_j exp(30*preds[i,j])
    den_i = exp(num_i) + S_i - exp(30*t_i)
    loss  = mean_i( log(den_i) - num_i )

Sharding: batch-parallel, 256 rows/core. Each row is rotated on the host so
its target column sits at local column 0 (row sums are rotation-invariant),
making the device program a pure streaming exp+rowsum with a tiny epilogue
and no gather / no collective. Final mean over the 8*[128,2] per-row losses
happens on the host (the "all-reduce" of the unshard step).

Schedule notes: the numerator chain (sqrt etc.) depends only on the target
column, so it is emitted first — its ACT table loads hide under the first
x-tile DMA. Deep x-tile buffering (bufs=8) keeps the DMA queue full so the
streaming phase is HBM-bound; ScalarE does exp + row-sum (accum_out) at
~1 elem/cycle/lane, below the DMA rate.
"""
import numpy as np
from contextlib import ExitStack

import concourse.bass as bass
import concourse.tile as tile
from concourse import bacc, mybir
from concourse.bass_utils import run_bass_kernel_spmd

B, V = 2048, 32000
N_CORES = 8
RPC = B // N_CORES            # 256 rows per core
P = 128                       # SBUF partitions
G = RPC // P                  # 2 row groups per core

# column tiling: small leading tiles (fast ScalarE start) for group 0,
# reversed for group 1 so the stream also ENDS on small tiles (short tail)
TILES = [500, 1500, 2000] + [4000] * 7
assert sum(TILES) == V
NT = len(TILES)
GTILES = [list(TILES), list(reversed(TILES))]

SCALE = 30.0
CM = SCALE * np.cos(0.5)      # 26.327476856711183
SM = SCALE * np.sin(0.5)      # 14.38276615812609
CB = SCALE * 0.35             # 10.5

F32 = mybir.dt.float32
AF = mybir.ActivationFunctionType
ALU = mybir.AluOpType

_cache = {}


def _build():
    nc = bacc.Bacc("TRN2", target_bir_lowering=False, debug=False,
                   num_devices=N_CORES)
    x = nc.dram_tensor("x", [RPC, V], F32, kind="ExternalInput")
    out = nc.dram_tensor("out", [P, G], F32, kind="ExternalOutput")

    with tile.TileContext(nc) as tc, ExitStack() as ctx:
        xpool = ctx.enter_context(tc.tile_pool(name="x", bufs=8))
        epool = ctx.enter_context(tc.tile_pool(name="e", bufs=2))
        spool = ctx.enter_context(tc.tile_pool(name="s", bufs=1))

        ssum = spool.tile([P, G * NT], F32)   # per-(group,tile) exp row-sums
        tvec = spool.tile([P, G], F32)        # target logits t

        # --- target column + sqrt chain, emitted pre-stream: ScalarE's table
        # order becomes [sqrt-set, exp-set] with both loads in the ramp shadow
        # (the remaining Exp epilogue ops come after the stream loop, reusing
        # the already-loaded exp set) ---
        with tc.high_priority():
            for g in range(G):
                nc.sync.dma_start(tvec[:, g:g + 1], x[g * P:(g + 1) * P, 0:1])

            tsq = spool.tile([P, G], F32)
            nc.vector.tensor_mul(tsq[:], tvec[:], tvec[:])
            omts = spool.tile([P, G], F32)
            # (t^2 * -1) + 1, clamped away from 0 for the sqrt
            nc.vector.tensor_scalar(omts[:], tsq[:], -1.0, 1.0,
                                    ALU.mult, ALU.add)
            omc = spool.tile([P, G], F32)
            nc.vector.tensor_scalar_max(omc[:], omts[:], 1e-30)
            r = spool.tile([P, G], F32)
            nc.scalar.activation(r[:], omc[:], AF.Sqrt)

        # --- streaming pass: exp(30 x) + per-row sums on ScalarE ---
        for g in range(G):
            rs = slice(g * P, (g + 1) * P)
            off = 0
            for t, tc_ in enumerate(GTILES[g]):
                xt = xpool.tile([P, tc_], F32, tag="xt")
                nc.sync.dma_start(xt[:], x[rs, off:off + tc_])
                et = epool.tile([P, tc_], F32, tag="et")
                nc.scalar.activation(
                    et[:], xt[:], AF.Exp, scale=SCALE,
                    accum_out=ssum[:, g * NT + t: g * NT + t + 1],
                )
                off += tc_

        # --- numerator epilogue (gap-fills into the stream; exp set stays) ---
        a = spool.tile([P, G], F32)
        nc.vector.tensor_scalar(a[:], tvec[:], float(CM), -float(CB),
                                ALU.mult, ALU.add)
        bb = spool.tile([P, G], F32)
        nc.vector.tensor_scalar_mul(bb[:], r[:], float(SM))
        num = spool.tile([P, G], F32)
        nc.vector.tensor_sub(num[:], a[:], bb[:])

        enum_ = spool.tile([P, G], F32)
        nc.scalar.activation(enum_[:], num[:], AF.Exp)
        e30t = spool.tile([P, G], F32)
        nc.scalar.activation(e30t[:], tvec[:], AF.Exp, scale=SCALE)
        # exp(num) - exp(30 t), folded before S arrives
        ed = spool.tile([P, G], F32)
        nc.vector.tensor_sub(ed[:], enum_[:], e30t[:])

        # --- tail: S, den, loss ---
        S = spool.tile([P, G], F32)
        for g in range(G):
            nc.vector.tensor_reduce(
                S[:, g:g + 1], ssum[:, g * NT:(g + 1) * NT],
                axis=mybir.AxisListType.X, op=ALU.add,
            )
        den = spool.tile([P, G], F32)
        nc.vector.tensor_add(den[:], S[:], ed[:])
        lden = spool.tile([P, G], F32)
        nc.scalar.activation(lden[:], den[:], AF.Ln)
        loss = spool.tile([P, G], F32)
        nc.vector.tensor_sub(loss[:], lden[:], num[:])

        nc.sync.dma_start(out[:, :], loss[:])

    nc.compile()
    return nc


def _get_nc():
    if "nc" not in _cache:
        _cache["nc"] = _build()
    return _cache["nc"]


def _shard(preds, labels):
    """Rotate each row so its target column lands at column 0; split by core."""
    preds = np.ascontiguousarray(preds, dtype=np.float32)
    labels = np.asarray(labels).astype(np.int64)
    in_maps = []
    for c in range(N_CORES):
        shard = np.empty((RPC, V), np.float32)
        for i in range(RPC):
            r = c * RPC + i
            l = int(labels[r])
            shard[i, :V - l] = preds[r, l:]
            shard[i, V - l:] = preds[r, :l]
        in_maps.append({"x": shard})
    return in_maps


def kernel(preds, labels):
    in_maps = _shard(preds, labels)
    nc = _get_nc()
    res = run_bass_kernel_spmd(nc, in_maps, list(range(N_CORES)))
    total = 0.0
    for c in range(N_CORES):
        total += np.asarray(res.results[c]["out"], np.float64).sum()
    return np.array(total / B, dtype=np.float32)


# revision 12
# speedup vs baseline: 1.0019x; 1.0019x over previous
"""CosArcLoss on 8 TRN2 NeuronCores (Bass/Tile).

Math (reference, f32):
    t_i   = preds[i, labels[i]]
    theta = arccos(clip(t_i, -1+1e-12, 1-1e-12))    # == clip(t_i,-1,1) in f32
    num_i = 30*(cos(theta + 0.5) - 0.35)
          = 30*cos(0.5)*t_i - 30*sin(0.5)*sqrt(1-t_i^2) - 10.5
    S_i   = sum_j exp(30*preds[i,j])
    den_i = exp(num_i) + S_i - exp(30*t_i)
    loss  = mean_i( log(den_i) - num_i )

Sharding: batch-parallel, 256 rows/core. Each row is rotated on the host so
its target column sits at local column 0 (row sums are rotation-invariant),
making the device program a pure streaming exp+rowsum with a tiny epilogue
and no gather / no collective. Final mean over the 8*[128,2] per-row losses
happens on the host (the "all-reduce" of the unshard step).

Schedule notes: the numerator chain (sqrt etc.) depends only on the target
column, so it is emitted first — its ACT table loads hide under the first
x-tile DMA. Deep x-tile buffering (bufs=8) keeps the DMA queue full so the
streaming phase is HBM-bound; ScalarE does exp + row-sum (accum_out) at
~1 elem/cycle/lane, below the DMA rate.
"""
import numpy as np
from contextlib import ExitStack

import concourse.bass as bass
import concourse.tile as tile
from concourse import bacc, mybir
from concourse.bass_utils import run_bass_kernel_spmd

B, V = 2048, 32000
N_CORES = 8
RPC = B // N_CORES            # 256 rows per core
P = 128                       # SBUF partitions
G = RPC // P                  # 2 row groups per core

# column tiling: small leading tiles (fast ScalarE start) for group 0,
# reversed for group 1 so the stream also ENDS on small tiles (short tail)
TILES = [500, 1500, 2000] + [4000] * 7
assert sum(TILES) == V
NT = len(TILES)
GTILES = [list(TILES), list(reversed(TILES))]

SCALE = 30.0
CM = SCALE * np.cos(0.5)      # 26.327476856711183
SM = SCALE * np.sin(0.5)      # 14.38276615812609
CB = SCALE * 0.35             # 10.5

F32 = mybir.dt.float32
AF = mybir.ActivationFunctionType
ALU = mybir.AluOpType

_cache = {}


def _build():
    nc = bacc.Bacc("TRN2", target_bir_lowering=False, debug=False,
                   num_devices=N_CORES)
    x = nc.dram_tensor("x", [RPC, V], F32, kind="ExternalInput")
    # out[:, 0:G] = den, out[:, G:2G] = num; the final ln(den)-num over the
    # 2048 per-row pairs happens host-side (saves the tail's ln-table load)
    out = nc.dram_tensor("out", [P, 2 * G], F32, kind="ExternalOutput")

    with tile.TileContext(nc) as tc, ExitStack() as ctx:
        xpool = ctx.enter_context(tc.tile_pool(name="x", bufs=8))
        epool = ctx.enter_context(tc.tile_pool(name="e", bufs=2))
        spool = ctx.enter_context(tc.tile_pool(name="s", bufs=1))

        ssum = spool.tile([P, G * NT], F32)   # per-(group,tile) exp row-sums
        tvec = spool.tile([P, G], F32)        # target logits t

        # --- target column + sqrt chain, emitted pre-stream: ScalarE's table
        # order becomes [sqrt-set, exp-set] with both loads in the ramp shadow
        # (the remaining Exp epilogue ops come after the stream loop, reusing
        # the already-loaded exp set) ---
        with tc.high_priority():
            for g in range(G):
                nc.sync.dma_start(tvec[:, g:g + 1], x[g * P:(g + 1) * P, 0:1])

            tsq = spool.tile([P, G], F32)
            nc.vector.tensor_mul(tsq[:], tvec[:], tvec[:])
            omts = spool.tile([P, G], F32)
            # (t^2 * -1) + 1, clamped away from 0 for the sqrt
            nc.vector.tensor_scalar(omts[:], tsq[:], -1.0, 1.0,
                                    ALU.mult, ALU.add)
            omc = spool.tile([P, G], F32)
            nc.vector.tensor_scalar_max(omc[:], omts[:], 1e-30)
            r = spool.tile([P, G], F32)
            nc.scalar.activation(r[:], omc[:], AF.Sqrt)

        # --- streaming pass: exp(30 x) + per-row sums on ScalarE ---
        for g in range(G):
            rs = slice(g * P, (g + 1) * P)
            off = 0
            for t, tc_ in enumerate(GTILES[g]):
                xt = xpool.tile([P, tc_], F32, tag="xt")
                nc.sync.dma_start(xt[:], x[rs, off:off + tc_])
                et = epool.tile([P, tc_], F32, tag="et")
                nc.scalar.activation(
                    et[:], xt[:], AF.Exp, scale=SCALE,
                    accum_out=ssum[:, g * NT + t: g * NT + t + 1],
                )
                off += tc_

        # --- numerator epilogue (gap-fills into the stream; exp set stays) ---
        a = spool.tile([P, G], F32)
        nc.vector.tensor_scalar(a[:], tvec[:], float(CM), -float(CB),
                                ALU.mult, ALU.add)
        bb = spool.tile([P, G], F32)
        nc.vector.tensor_scalar_mul(bb[:], r[:], float(SM))
        num = spool.tile([P, G], F32)
        nc.vector.tensor_sub(num[:], a[:], bb[:])

        enum_ = spool.tile([P, G], F32)
        nc.scalar.activation(enum_[:], num[:], AF.Exp)
        e30t = spool.tile([P, G], F32)
        nc.scalar.activation(e30t[:], tvec[:], AF.Exp, scale=SCALE)
        # exp(num) - exp(30 t), folded before S arrives
        ed = spool.tile([P, G], F32)
        nc.vector.tensor_sub(ed[:], enum_[:], e30t[:])

        # --- tail: S, den, loss ---
        S = spool.tile([P, G], F32)
        for g in range(G):
            nc.vector.tensor_reduce(
                S[:, g:g + 1], ssum[:, g * NT:(g + 1) * NT],
                axis=mybir.AxisListType.X, op=ALU.add,
            )
        dn = spool.tile([P, 2 * G], F32)
        nc.vector.tensor_add(dn[:, 0:G], S[:], ed[:])
        nc.vector.tensor_copy(dn[:, G:2 * G], num[:])

        nc.sync.dma_start(out[:, :], dn[:])

    nc.compile()
    return nc


def _get_nc():
    if "nc" not in _cache:
        _cache["nc"] = _build()
    return _cache["nc"]


def _shard(preds, labels):
    """Rotate each row so its target column lands at column 0; split by core."""
    preds = np.ascontiguousarray(preds, dtype=np.float32)
    labels = np.asarray(labels).astype(np.int64)
    in_maps = []
    for c in range(N_CORES):
        shard = np.empty((RPC, V), np.float32)
        for i in range(RPC):
            r = c * RPC + i
            l = int(labels[r])
            shard[i, :V - l] = preds[r, l:]
            shard[i, V - l:] = preds[r, :l]
        in_maps.append({"x": shard})
    return in_maps


def kernel(preds, labels):
    in_maps = _shard(preds, labels)
    nc = _get_nc()
    res = run_bass_kernel_spmd(nc, in_maps, list(range(N_CORES)))
    total = 0.0
    for c in range(N_CORES):
        o = np.asarray(res.results[c]["out"], np.float64)
        den, num = o[:, :G], o[:, G:]
        total += (np.log(den) - num).sum()
    return np.array(total / B, dtype=np.float32)
